# revision 15
# baseline (speedup 1.0000x reference)
"""Trainium2 Bass kernel for CuteInferMLP: E = gelu(X @ W0^T + b0) @ W1^T + b1.

Full shapes: x (2, 2048, 2048) f32, W0 (8192, 2048), b0 (8192,),
W1 (2048, 8192), b1 (2048,). Output (2, 2048, 2048) f16.

Sharding: 8-way data-parallel over the 4096 tokens (512 tokens/core).
Each core holds the full (fp16-cast) weights and computes its token
slice end to end; the host just concatenates the 8 slices.

Device layout per core (all matmuls keep weights stationary on the PE,
contraction dim on partitions):
  GEMM0: D^T[n,m] += W0T[h,n]^T-tile-stationary @ X^T[h,m]   (h = 16 k-tiles)
  act:   D^T = gelu(D^T + b0)  (ScalarE, fused bias + fp16 cast)
  GEMM1: E^T[hh,m] += W1T[n,hh]-stationary @ D^T[n,m]        (n = 64 k-tiles)
  act:   E^T = E^T + b1 (Identity activation, fp16 cast), DMA out.

The steady state runs at the PE issue floor (one 128x128x512 matmul per
216 ns); the remaining overhead is all at the head, where the first
matmuls wait on input DMA and run at the HAM-throttled 1.2 GHz clock.
Three measures address it:
  - 18 warmup matmuls on a zeroed scratch tile keep the PE busy from
    engine-init until real inputs land, so the HAM clock-gate reaches
    8/8 (2.4 GHz) before the first real matmul and never re-throttles.
  - The first PREF=6 output blocks of GEMM0 are computed kb-major into
    6 parallel PSUM banks: each step kb touches only x k-tile kb
    (128 KB) and a [128, 6, 128] weight sliver (192 KB), so matmuls
    start as soon as those land instead of waiting for all of x (2 MB).
    Weight slivers for this prefix use a kb-major DRAM layout (w0a).
  - x streams in 5 geometric chunks (1/1/2/4/8 k-tiles) interleaved
    with the weight slivers on the sync queue so the earliest
    dependencies have the shortest descriptor + transfer latency.
"""

import numpy as np

from concourse import bacc, tile, mybir
from concourse.bass_utils import run_bass_kernel_spmd

P = 128
N_CORES = 8
B, L, H, N = 2, 2048, 2048, 8192
M = B * L                 # 4096 tokens
M_CORE = M // N_CORES     # 512 tokens per core
KB0 = H // P              # 16  k-tiles in GEMM0 (contraction over H)
NB = N // P               # 64  n-blocks (GEMM0 output partitions)
KB1 = N // P              # 64  k-tiles in GEMM1 (contraction over N)
HB = H // P               # 16  output blocks (GEMM1 output partitions)
PREF = 8                  # GEMM0 n-blocks computed kb-major at the head
WARM_MM = 9               # warmup matmuls (~3.8 us of PE busy at 1.2 GHz)

TRACE = False             # set True by test harness for NTFF profiling
LAST_EXEC_NS = None       # populated when TRACE

_CACHED = {}


def _build_nc():
    fp16 = mybir.dt.float16
    f32 = mybir.dt.float32
    gelu = mybir.ActivationFunctionType.Gelu
    ident = mybir.ActivationFunctionType.Identity

    nc = bacc.Bacc("TRN2", target_bir_lowering=False, debug=False,
                   num_devices=N_CORES)
    xT = nc.declare_dram_parameter("xT", [P, KB0, M_CORE], fp16, isOutput=False)
    w0a = nc.declare_dram_parameter("w0a", [KB0, P, PREF, P], fp16,
                                    isOutput=False)
    w0b = nc.declare_dram_parameter("w0b", [NB - PREF, P, KB0, P], fp16,
                                    isOutput=False)
    w1 = nc.declare_dram_parameter("w1", [HB, P, KB1, P], fp16, isOutput=False)
    b0 = nc.declare_dram_parameter("b0", [P, NB], f32, isOutput=False)
    b1 = nc.declare_dram_parameter("b1", [P, HB], f32, isOutput=False)
    out = nc.declare_dram_parameter("out", [HB, P, M_CORE], fp16, isOutput=True)

    with tile.TileContext(nc) as tc:
        with (
            tc.tile_pool(name="sb", bufs=1) as sb_pool,
            tc.tile_pool(name="psp", bufs=1, space="PSUM") as ps_pool,
        ):
            # Warmup: keep the PE busy from engine-init until the first
            # real matmul's inputs land, so the HAM clock-gate opens to
            # 2.4 GHz before real work starts.
            warm_sb = sb_pool.tile([P, M_CORE], fp16)
            nc.vector.memset(warm_sb[:], 0)
            ps_w = ps_pool.tile([P, M_CORE], f32, name="ps_warm", tag="ps",
                                bufs=PREF)
            for _ in range(WARM_MM):
                nc.tensor.matmul(ps_w[:], lhsT=warm_sb[:, 0:P],
                                 rhs=warm_sb[:], start=True, stop=True)

            # Head DMAs on sync, earliest dependencies first: a 1-k-tile
            # x chunk and the first weight sliver, then progressively
            # larger x chunks interleaved with more slivers.
            x_sb = sb_pool.tile([P, KB0, M_CORE], fp16)
            d_sb = sb_pool.tile([P, KB1, M_CORE], fp16)

            w0a_tiles = []

            def w0a_load(kb):
                t = sb_pool.tile([P, PREF, P], fp16, name="w0a_sb",
                                 tag="w0a_sb", bufs=6)
                nc.sync.dma_start(out=t[:], in_=w0a[kb])
                w0a_tiles.append(t)

            def x_load(k0, k1):
                nc.sync.dma_start(out=x_sb[:, k0:k1, :], in_=xT[:, k0:k1, :])

            x_load(0, 1)
            w0a_load(0)
            x_load(1, 2)
            w0a_load(1)
            x_load(2, 3)
            w0a_load(2)
            x_load(3, 4)
            w0a_load(3)
            x_load(4, 8)
            w0a_load(4)
            w0a_load(5)
            w0a_load(6)
            w0a_load(7)
            x_load(8, 12)
            w0a_load(8)
            w0a_load(9)
            w0a_load(10)
            x_load(12, KB0)
            for kb in range(11, KB0):
                w0a_load(kb)
            del x_load
            b0_sb = sb_pool.tile([P, NB], f32)
            nc.sync.dma_start(out=b0_sb[:], in_=b0[:])
            b1_sb = sb_pool.tile([P, HB], f32)
            nc.sync.dma_start(out=b1_sb[:], in_=b1[:])

            # GEMM0 prefix: n-blocks 0..PREF-1, kb-major into PREF
            # parallel PSUM banks.  Step kb touches only x k-tile kb and
            # one weight sliver, so the PE starts as soon as those land.
            ps_list = [ps_pool.tile([P, M_CORE], f32, name=f"ps_pre{j}",
                                    tag="ps", bufs=PREF) for j in range(PREF)]
            for kb in range(KB0):
                for j in range(PREF):
                    nc.tensor.matmul(
                        ps_list[j][:],
                        lhsT=w0a_tiles[kb][:, j, :],
                        rhs=x_sb[:, kb, :],
                        start=(kb == 0),
                        stop=(kb == KB0 - 1),
                    )
            for j in range(PREF):
                nc.scalar.activation(
                    d_sb[:, j, :], ps_list[j][:], gelu,
                    bias=b0_sb[:, j:j + 1], scale=1.0,
                )

            # GEMM0 remainder: n-blocks PREF..NB-1, block-major with the
            # weight stream paced by pool slots.
            for nb in range(PREF, NB):
                w0_sb = sb_pool.tile([P, KB0, P], fp16, name="w0_sb",
                                     tag="w0_sb", bufs=6)
                nc.sync.dma_start(out=w0_sb[:], in_=w0b[nb - PREF])
                ps = ps_pool.tile([P, M_CORE], f32, tag="ps", bufs=PREF)
                for kb in range(KB0):
                    nc.tensor.matmul(
                        ps[:],
                        lhsT=w0_sb[:, kb, :],
                        rhs=x_sb[:, kb, :],
                        start=(kb == 0),
                        stop=(kb == KB0 - 1),
                    )
                nc.scalar.activation(
                    d_sb[:, nb, :], ps[:], gelu,
                    bias=b0_sb[:, nb:nb + 1], scale=1.0,
                )

            # GEMM1 + bias -> E^T, streamed out
            for hb in range(HB):
                w1_sb = sb_pool.tile([P, KB1, P], fp16, name="w1_sb",
                                     tag="w1_sb", bufs=3)
                nc.sync.dma_start(out=w1_sb[:], in_=w1[hb])
                ps = ps_pool.tile([P, M_CORE], f32, tag="ps", bufs=PREF)
                for kb in range(KB1):
                    nc.tensor.matmul(
                        ps[:],
                        lhsT=w1_sb[:, kb, :],
                        rhs=d_sb[:, kb, :],
                        start=(kb == 0),
                        stop=(kb == KB1 - 1),
                    )
                o_sb = sb_pool.tile([P, M_CORE], fp16, name="o_sb",
                                    tag="o_sb", bufs=4)
                nc.scalar.activation(
                    o_sb[:], ps[:], ident,
                    bias=b1_sb[:, hb:hb + 1], scale=1.0,
                )
                nc.sync.dma_start(out=out[hb], in_=o_sb[:])

    nc.compile()
    return nc


def kernel(x, W0, bias0, W1, bias1):
    global LAST_EXEC_NS

    if "nc" not in _CACHED:
        _CACHED["nc"] = _build_nc()
    nc = _CACHED["nc"]

    x, W0, bias0, W1, bias1 = (
        np.asarray(t) for t in (x, W0, bias0, W1, bias1))
    X = np.ascontiguousarray(x.reshape(M, H)).astype(np.float16)
    W0h = W0.astype(np.float16)
    w0a_host = np.ascontiguousarray(
        W0h[:PREF * P].reshape(PREF, P, KB0, P).transpose(2, 3, 0, 1))
    w0b_host = np.ascontiguousarray(
        W0h[PREF * P:].reshape(NB - PREF, P, KB0, P).transpose(0, 3, 2, 1))
    w1_host = np.ascontiguousarray(
        W1.astype(np.float16).reshape(HB, P, KB1, P).transpose(0, 3, 2, 1))
    b0_host = np.ascontiguousarray(bias0.astype(np.float32).reshape(NB, P).T)
    b1_host = np.ascontiguousarray(bias1.astype(np.float32).reshape(HB, P).T)

    in_maps = []
    for c in range(N_CORES):
        xs = X[c * M_CORE:(c + 1) * M_CORE]          # (512, 2048)
        xT_host = np.ascontiguousarray(
            xs.T.reshape(KB0, P, M_CORE).transpose(1, 0, 2))
        in_maps.append({
            "xT": xT_host, "w0a": w0a_host, "w0b": w0b_host, "w1": w1_host,
            "b0": b0_host, "b1": b1_host,
        })

    res = run_bass_kernel_spmd(
        nc, in_maps, core_ids=list(range(N_CORES)), trace=TRACE)
    if TRACE:
        LAST_EXEC_NS = res.exec_time_ns

    E = np.empty((M, H), dtype=np.float16)
    for c in range(N_CORES):
        o = res.results[c]["out"]                    # (HB, P, M_CORE)
        E[c * M_CORE:(c + 1) * M_CORE] = o.transpose(2, 0, 1).reshape(M_CORE, H)
    return E.reshape(B, L, H)
